# revision 22
# baseline (speedup 1.0000x reference)
"""Multi-head attention (B=2, S=2048, RES=1024, H=16) on 8 NeuronCores.

Sharding: batch*heads across cores. Core c handles batch c//4 and heads
4*(c%4) .. 4*(c%4)+3 (column-sharded QKV weights). No cross-core comm.

ACT-saturation design: the softmax exp is ACT-engine-bound (~1.0us per
[128,1024] tile, 128 tiles = ~130us busy) and total PE matmul work is
~160us; the schedule keeps both engines dense:
  - x arrives HOST-TRANSPOSED (xT [K, S]); W arrives host-relaid as
    [128, 8*256] (4KB partition lines -> descriptor-cheap DMAs).
  - phase-1 prologue rides the input DMA: 11 concurrent PSUM accumulators
    (K01/Q01 projections + V tiles 0..7 packed 4-per-psum-pair) consume
    each xt chunk the moment it lands; all 8 PSUM banks in use.
  - remaining projections (K01 s1-3, V8..15, Q01 s2/3, all of heads 2/3)
    are aux tasks popped one per iteration inside the attention loop,
    ordered by hard producer-before-consumer emission deadlines.
  - one flat 128-iteration pipeline (8 head/s-half groups x 16 t-blocks)
    processed in PAIRS: QK matmul pairs for two iterations back-to-back
    (one PE 64-row tiling-mode drain per pair instead of two), then both
    exps on ACT, then PV pairs lagged PVLAG iterations so PV matmuls
    never reach the PE queue head before their `at` input is ready.
  - group order (h0,s0),(h1,s0),(h0,s1),(h1,s1),(h2,s0),(h3,s0),(h2,s1),
    (h3,s1): h2/h3 projections have until iteration 64 to complete as aux
    work, and the s0 output tiles finalize/DMA mid-loop.
  - output is bf16 (host upcasts): halves the out-DMA tail; out tiles
    live in one [128, 16*C] tile so each s-half drains in a single
    large-line DMA.

Per-core kernel (S=2048, K=1024, C=256 = 4 heads x 64), bf16 matmuls
with fp32 PSUM accumulation:
  QT = (Wq_c)^T x^T  [C, S]      K on partitions (xT direct from DRAM)
  KT = (Wk_c)^T x^T  [C, S]
  V  = x Wv_c        [S, C] (+ ones col per head -> softmax sums ride
                             along in the PV matmul)
  per head: scoresT[t,s] = K_h^T Q_h -> exp(x/8) on ACT -> attnT (bf16)
            outT[d,s] (+ sums row) = V_aug^T attnT  (fp32 psum, 16 t-blocks)
            DMA xbar-transpose outT back to [s, d] in halves,
            rows * 1/sums, DMA out per s-half.

Q^T/K^T are stored per head with the 64 d-rows duplicated into partitions
64:128 so the two 512-wide QK matmuls of an iteration go to distinct PE
row groups. PSUM note: start=True clears has_written for a whole bank, so
packed accumulators only issue it on the first region per bank.
"""

import sys

if "/opt/trn_rl_repo" not in sys.path:
    sys.path.insert(0, "/opt/trn_rl_repo")

import numpy as np

B = 2
S = 2048
RES = 1024
HEADS = 16
HD = 64  # head dim
N_CORES = 8
HPC = 4  # heads per core
C = HPC * HD  # 256 per-core projected width
K = RES  # contraction dim of projections
NKT = K // 128  # 8 k-chunks
NST = S // 128  # 16 s-tiles / t-blocks
SH = 1024  # s-half size for attention inner loop
VAUG = HD + 2  # 66: V cols + ones col + zero pad
PVLAG = 8  # PV consumes at from PVLAG iterations ago

_CACHE: dict = {}


def _build_nc():
    import concourse.mybir as mybir
    import concourse.tile as tile
    from concourse import bacc
    from concourse.masks import make_identity

    f32 = mybir.dt.float32
    bf16 = mybir.dt.bfloat16
    AF = mybir.ActivationFunctionType

    nc = bacc.Bacc(None)
    xt_in = nc.dram_tensor("xt", [K, S], bf16, kind="ExternalInput")
    # W arrives host-relaid as [128, NKT*C]: w[p, kk*C+c] = W[kk*128+p, c]
    # -> 4KB contiguous per partition, one descriptor-cheap DMA per tensor
    wq_in = nc.dram_tensor("wq", [128, NKT * C], bf16, kind="ExternalInput")
    wk_in = nc.dram_tensor("wk", [128, NKT * C], bf16, kind="ExternalInput")
    wv_in = nc.dram_tensor("wv", [128, NKT * C], bf16, kind="ExternalInput")
    out_d = nc.dram_tensor("out", [S, C], bf16, kind="ExternalOutput")

    # group order: (h, shi). h2/h3 delayed so their projections can be aux
    # work; s0 appears in the first half for both head pairs so its out
    # tiles finalize mid-loop.
    GROUPS = [(0, 0), (1, 0), (0, 1), (1, 1), (2, 0), (3, 0), (2, 1), (3, 1)]
    NITER = len(GROUPS) * NST  # 128

    with tile.TileContext(nc) as tc:
        with (
            tc.tile_pool(name="persist", bufs=1) as persist,
            tc.tile_pool(name="attn", bufs=2) as attn,
            tc.tile_pool(name="psum", bufs=1, space="PSUM") as ps,
        ):
            ident32 = persist.tile([128, 128], f32)
            make_identity(nc, ident32)
            ident = persist.tile([128, 128], bf16)
            nc.vector.tensor_copy(ident[:], ident32[:])
            ones4 = persist.tile([128, HPC], f32)
            nc.vector.memset(ones4[:], 1.0)
            zeros4 = persist.tile([128, HPC], f32)
            nc.vector.memset(zeros4[:], 0.0)

            # ---- input DMAs ----
            wq_t, wk_t, wv_t = [], [], []
            for lst, src, nm in ((wq_t, wq_in, "wq"), (wk_t, wk_in, "wk"),
                                 (wv_t, wv_in, "wv")):
                w_all = persist.tile([128, NKT * C], bf16, name=f"{nm}_all")
                nc.gpsimd.dma_start(w_all[:], src[:, :])
                for kk in range(NKT):
                    lst.append(w_all[:, kk * C:(kk + 1) * C])
            xt_sb = []
            for kk in range(NKT):
                t_ = persist.tile([128, S], bf16, name=f"xt_{kk}", tag="xt",
                                  bufs=NKT)
                nc.sync.dma_start(t_[:], xt_in[kk * 128:(kk + 1) * 128, :])
                xt_sb.append(t_)

            # per-head Q^T/K^T with the head's 64 d-rows duplicated into
            # partitions 64:128 (distinct PE row groups for the QK pair)
            qt_tiles = []
            kt_tiles = []
            for h in range(HPC):
                qt = persist.tile([128, S], bf16, name=f"qt_{h}", tag="qt",
                                  bufs=HPC)
                kt = persist.tile([128, S], bf16, name=f"kt_{h}", tag="kt",
                                  bufs=HPC)
                qt_tiles.append(qt)
                kt_tiles.append(kt)

            v_aug = []
            for st in range(NST):
                va = persist.tile([128, HPC * VAUG], bf16, name=f"vaug_{st}",
                                  tag="vaug", bufs=NST)
                v_aug.append(va)

            out_big = persist.tile([128, NST * C], bf16, name="out_big")
            out_tiles = [out_big[:, sb * C:(sb + 1) * C] for sb in range(NST)]
            # DRAM view: row sb*128+p, col c  <->  sbuf [p, sb*C + c]
            out_v = out_d.rearrange("(j p) c -> p j c", p=128)

            # ---- helpers ----
            def emit_warm(n):
                # tiny full-array matmuls into an sc-tagged psum tile to keep
                # the PE clock-gate warm; results never read.
                wm = ps.tile([128, SH], f32, name="warm", tag="sc", bufs=2)
                for w in range(n):
                    nc.tensor.matmul(
                        wm[:, (w % 8) * 64:(w % 8) * 64 + 64],
                        ident[:], ident[:, 0:64],
                        start=True, stop=True, skip_group_check=True,
                    )

            def finish_projqk(pp, dsts, half, sc, use_act=False):
                # use_act: offload half the duplication copies to the (idle)
                # scalar engine -- prologue only; during the loop ACT is the
                # bottleneck and must not be stolen from.
                stg = attn.tile([128, 512], bf16, name=f"stg_{half}_{sc}",
                                tag="stg", bufs=2)
                nc.vector.tensor_copy(stg[:], pp[:])
                cols = slice(sc * 512, (sc + 1) * 512)
                for hh in range(2):
                    dst = dsts[2 * half + hh]
                    nc.vector.tensor_copy(dst[0:HD, cols],
                                          stg[hh * HD:(hh + 1) * HD, :])
                    if use_act:
                        nc.scalar.copy(dst[HD:128, cols],
                                       stg[hh * HD:(hh + 1) * HD, :])
                    else:
                        nc.vector.tensor_copy(dst[HD:128, cols],
                                              stg[hh * HD:(hh + 1) * HD, :])

            def emit_projqk(w_t, dsts, half, sc):
                # one 512-col chunk of a Q/K projection for a head pair.
                # half=0 -> heads 0,1 ; half=1 -> heads 2,3
                pp = ps.tile([128, 512], f32, name=f"pp_{half}_{sc}", tag="pp",
                             bufs=2)
                for kk in range(NKT):
                    nc.tensor.matmul(
                        pp[:],
                        w_t[kk][:, half * 128:half * 128 + 128],
                        xt_sb[kk][:, sc * 512:(sc + 1) * 512],
                        start=(kk == 0),
                        stop=(kk == NKT - 1),
                    )
                finish_projqk(pp, dsts, half, sc)

            def emit_vproj(st):
                va3 = v_aug[st].rearrange("p (h d) -> p h d", h=HPC)
                vp = ps.tile([128, C], f32, name=f"vp_{st}", tag="pp", bufs=2)
                for kk in range(NKT):
                    nc.tensor.matmul(
                        vp[:],
                        xt_sb[kk][:, st * 128:(st + 1) * 128],
                        wv_t[kk][:],
                        start=(kk == 0),
                        stop=(kk == NKT - 1),
                    )
                nc.vector.tensor_copy(
                    va3[:, :, 0:HD], vp.rearrange("p (h d) -> p h d", h=HPC))
                nc.vector.tensor_copy(
                    va3[:, :, HD:HD + 1],
                    ones4.rearrange("p (h o) -> p h o", h=HPC))
                nc.vector.tensor_copy(
                    va3[:, :, HD + 1:HD + 2],
                    zeros4.rearrange("p (h o) -> p h o", h=HPC))

            # ---- prologue: phase-1 DMA tracking ----
            # 11 concurrent psum accumulators ride the half-0 xt DMA:
            # K01s0+Q01s0 (pp), Q01s1 (outT), V0..3 / V4..7 packed into the
            # two sc-tag buffers. Every matmul runs as soon as chunk kk lands.
            emit_warm(16)
            ppk = ps.tile([128, 512], f32, name="ppk0", tag="pp", bufs=2)
            ppq = ps.tile([128, 512], f32, name="ppq0", tag="pp", bufs=2)
            ppq1 = ps.tile([128, 512], f32, name="ppq1", tag="outT", bufs=1)
            packA = ps.tile([128, SH], f32, name="vpackA", tag="sc", bufs=2)
            packB = ps.tile([128, SH], f32, name="vpackB", tag="sc", bufs=2)
            for kk in range(NKT):
                st_ = (kk == 0)
                sp_ = (kk == NKT - 1)
                nc.tensor.matmul(ppk[:], wk_t[kk][:, 0:128],
                                 xt_sb[kk][:, 0:512], start=st_, stop=sp_)
                nc.tensor.matmul(ppq[:], wq_t[kk][:, 0:128],
                                 xt_sb[kk][:, 0:512], start=st_, stop=sp_)
                nc.tensor.matmul(ppq1[:], wq_t[kk][:, 0:128],
                                 xt_sb[kk][:, 512:1024], start=st_, stop=sp_)
                # PSUM start=True clears has_written for the WHOLE bank, so
                # only the first region of each 512-f32 bank may issue it; the
                # second region (odd j) relies on the cleared bits: its first
                # matmul overwrites, later ones accumulate.
                for j in range(4):
                    nc.tensor.matmul(
                        packA[:, j * C:(j + 1) * C],
                        xt_sb[kk][:, j * 128:(j + 1) * 128], wv_t[kk][:],
                        start=st_ and (j % 2 == 0), stop=sp_,
                        skip_group_check=True)
                for j in range(4):
                    nc.tensor.matmul(
                        packB[:, j * C:(j + 1) * C],
                        xt_sb[kk][:, (4 + j) * 128:(5 + j) * 128], wv_t[kk][:],
                        start=st_ and (j % 2 == 0), stop=sp_,
                        skip_group_check=True)
            finish_projqk(ppk, kt_tiles, 0, 0, use_act=True)
            finish_projqk(ppq, qt_tiles, 0, 0, use_act=True)
            finish_projqk(ppq1, qt_tiles, 0, 1, use_act=True)

            def finish_vproj(vp, st):
                va3 = v_aug[st].rearrange("p (h d) -> p h d", h=HPC)
                nc.vector.tensor_copy(
                    va3[:, :, 0:HD], vp.rearrange("p (h d) -> p h d", h=HPC))
                nc.vector.tensor_copy(
                    va3[:, :, HD:HD + 1],
                    ones4.rearrange("p (h o) -> p h o", h=HPC))
                nc.vector.tensor_copy(
                    va3[:, :, HD + 1:HD + 2],
                    zeros4.rearrange("p (h o) -> p h o", h=HPC))

            for j in range(4):
                finish_vproj(packA[:, j * C:(j + 1) * C], j)
            for j in range(4):
                finish_vproj(packB[:, j * C:(j + 1) * C], 4 + j)

            # ---- aux work queue (popped one per iteration) ----
            # K01s1..3 use half-0 xt (ready). V8..15 and Q01s2/3 need half-1
            # chunks; they are queued late enough that their data has mostly
            # arrived (slots 8..17), but still before their PV consumers.
            aux = [None] * 4
            aux[0] = lambda: emit_projqk(wk_t, kt_tiles, 0, 1)
            aux[1] = lambda: emit_projqk(wk_t, kt_tiles, 0, 2)
            aux[2] = lambda: emit_projqk(wk_t, kt_tiles, 0, 3)
            # V8..15 every other slot (PVLAG=8 relaxes their deadlines to
            # iteration t+8, so the per-iteration aux load stays ~0.5us)
            for st in range(8, NST):
                aux.append(lambda st=st: emit_vproj(st))
                aux.append(None)
            for task in (
                lambda: emit_projqk(wq_t, qt_tiles, 0, 2),
                lambda: emit_projqk(wq_t, qt_tiles, 0, 3),
                lambda: emit_projqk(wk_t, kt_tiles, 1, 0),
                lambda: emit_projqk(wk_t, kt_tiles, 1, 1),
                lambda: emit_projqk(wq_t, qt_tiles, 1, 0),
                lambda: emit_projqk(wk_t, kt_tiles, 1, 2),
                lambda: emit_projqk(wk_t, kt_tiles, 1, 3),
                lambda: emit_projqk(wq_t, qt_tiles, 1, 1),
                lambda: emit_projqk(wq_t, qt_tiles, 1, 2),
                lambda: emit_projqk(wq_t, qt_tiles, 1, 3),
            ):
                aux.append(task)
                aux.append(None)

            # ---- attention pipeline ----
            pending = []  # (g, t, at_tile)
            outp_of = {}  # g -> psum tile
            tails = {0: [], 1: []}  # shi -> finished (h, oT) for normalize
            done_heads = {0: 0, 1: 0}

            def emit_pv(g, t, at):
                h, shi = GROUPS[g]
                if t == 0:
                    outp_of[g] = ps.tile([VAUG, SH], f32, name=f"outT_{g}",
                                         tag="outT", bufs=1)
                outp = outp_of[g]
                for scj in range(SH // 512):
                    nc.tensor.matmul(
                        outp[:, scj * 512:(scj + 1) * 512],
                        v_aug[t][:, h * VAUG:(h + 1) * VAUG],
                        at[:, scj * 512:(scj + 1) * 512],
                        start=(t == 0),
                        stop=(t == NST - 1),
                    )
                if t == NST - 1:
                    oT = attn.tile([80, SH], bf16, name=f"oT_{g}", tag="oT",
                                   bufs=4)
                    outp_done = outp_of.pop(g)
                    # copy in halves so the first starts while the second
                    # PV chunk is still streaming
                    nc.vector.tensor_copy(oT[0:VAUG, 0:512],
                                          outp_done[:, 0:512])
                    nc.vector.tensor_copy(oT[0:VAUG, 512:SH],
                                          outp_done[:, 512:SH])
                    emit_tail(g, oT)

            def emit_tail(g, oT):
                # DMA xbar transpose back to [s, d] (in halves, pipelined
                # with the normalize), then normalize rows by 1/sums
                # (col HD of the transposed block)
                h, shi = GROUPS[g]
                trb = attn.tile([128, (SH // 128) * 80], bf16,
                                name=f"trb_{g}", tag="trb", bufs=4)
                trb3 = trb.rearrange("p (j c) -> p j c", j=SH // 128)
                for jh in range(2):
                    jlo, jhi = jh * 4, jh * 4 + 4
                    nc.sync.dma_start_transpose(
                        trb3[:, jlo:jhi, :], oT[0:80, jlo * 128:jhi * 128])
                    for j in range(jlo, jhi):
                        sb = shi * (SH // 128) + j
                        rs = attn.tile([128, 1], f32, name=f"rs_{g}_{j}",
                                       tag="rs", bufs=8)
                        nc.vector.reciprocal(rs[:], trb3[:, j, HD:HD + 1])
                        nc.vector.tensor_scalar_mul(
                            out_tiles[sb][:, h * HD:(h + 1) * HD],
                            trb3[:, j, 0:HD], rs[:])
                done_heads[shi] += 1
                if done_heads[shi] == HPC:
                    # whole s-half finished: one large-line DMA for its 8
                    # out tiles (sb = shi*8 .. shi*8+7)
                    j0 = shi * (SH // 128)
                    nc.sync.dma_start(
                        out_v[:, j0:j0 + SH // 128, :],
                        out_big.rearrange("p (j c) -> p j c", c=C)[
                            :, j0:j0 + SH // 128, :])

            def emit_qk(it):
                # returns (g, t, at) for the pending-PV queue
                g, t = divmod(it, NST)
                h, shi = GROUPS[g]
                qt = qt_tiles[h]
                kt = kt_tiles[h]
                s0 = shi * SH
                sc_ps = ps.tile([128, SH], f32, name=f"sc_{it}", tag="sc",
                                bufs=2)
                for scj in range(SH // 512):
                    dlo = scj * HD
                    dhi = dlo + HD
                    nc.tensor.matmul(
                        sc_ps[:, scj * 512:(scj + 1) * 512],
                        kt[dlo:dhi, t * 128:(t + 1) * 128],
                        qt[dlo:dhi, s0 + scj * 512:s0 + (scj + 1) * 512],
                        start=True, stop=True, skip_group_check=True,
                    )
                at = attn.tile([128, SH], bf16, name=f"at_{it}", tag="at",
                               bufs=PVLAG + 2)
                return g, t, at, sc_ps

            # iterations processed in PAIRS: both QK matmul pairs (64-row
            # tiling mode) back-to-back, then both exps, then aux + PV
            # (128-row mode) -- one PE tiling-mode drain per pair instead
            # of two per iteration.
            for p in range(NITER // 2):
                q0 = emit_qk(2 * p)
                q1 = emit_qk(2 * p + 1)
                for (g, t, at, sc_ps) in (q0, q1):
                    nc.scalar.activation(at[:], sc_ps[:], AF.Exp, scale=0.125)
                    pending.append((g, t, at))
                for _ in range(2):
                    if aux:
                        fn = aux.pop(0)
                        if fn is not None:
                            fn()
                lag = 2 if p >= NITER // 2 - 8 else PVLAG
                while len(pending) > lag:
                    emit_pv(*pending.pop(0))
            while pending:
                emit_pv(*pending.pop(0))

    nc.finalize()
    return nc


def _get_nc():
    if "nc" not in _CACHE:
        _CACHE["nc"] = _build_nc()
    return _CACHE["nc"]


def kernel(x, Wq, Wk, Wv):
    import ml_dtypes
    from concourse import bass_utils

    bf = ml_dtypes.bfloat16
    x = np.asarray(x, dtype=np.float32).astype(bf)
    Wq = np.asarray(Wq, dtype=np.float32).astype(bf)
    Wk = np.asarray(Wk, dtype=np.float32).astype(bf)
    Wv = np.asarray(Wv, dtype=np.float32).astype(bf)

    nc = _get_nc()

    def relay(W, cols):
        # [1024, 256] -> [128, 8*256]: w[p, kk*C+c] = W[kk*128+p, c]
        return np.ascontiguousarray(
            W[:, cols].reshape(NKT, 128, C).transpose(1, 0, 2).reshape(
                128, NKT * C))

    in_maps = []
    for c in range(N_CORES):
        b = c // 4
        g = c % 4
        cols = slice(g * C, (g + 1) * C)
        in_maps.append(
            {
                "xt": np.ascontiguousarray(x[b].T),
                "wq": relay(Wq, cols),
                "wk": relay(Wk, cols),
                "wv": relay(Wv, cols),
            }
        )

    res = bass_utils.run_bass_kernel_spmd(nc, in_maps, list(range(N_CORES)))
    _CACHE["last_results"] = res

    out = np.empty((B, S, RES), dtype=np.float32)
    for c in range(N_CORES):
        b = c // 4
        g = c % 4
        out[b, :, g * C : (g + 1) * C] = res.results[c]["out"].astype(np.float32)
    return out
